# revision 25
# baseline (speedup 1.0000x reference)
"""GCN (2-layer, PyG gcn_norm) on 8 Trainium2 NeuronCores via Bass.

Strategy (dst-partition-row sharding, no collectives, no PE):
  * Host appends self-loop edges (weight 1, as in gcn_norm), sorts nodes
    by in-degree and assigns each node one SBUF partition-row of G slots
    (G = per-stripe max degree rounded up to 8; ~12% padding), so the
    per-node segment-sum needs no one-hot masks or matmuls.  Stripes of
    1024 nodes (one 128-node tile per core) share a G schedule so all 8
    SPMD cores run one program.
  * Per-edge streams are bf16 (tolerance is 2e-2).  The segment-sum runs
    as a packed-bf16 pair-add tree on DVE (tensor_tensor at the 2x rate;
    TensorReduce itself has no fast mode) with a final narrow f32
    tensor_reduce per equal-G run.  The layer-1 node epilogue
    (z->W1->relu->W2) uses weight tiles pre-materialized in (hidden,
    tile) layout so every product is a packed 2x tensor_tensor.
  * Streams transfer as NBLK whole contiguous blocks per sweep, each a
    single full-width dma_start round-robined across the two hardware
    DGE queues (SP + Activation); the Activation engine does no stream
    compute so its queue can prefetch.  Per-node output flushes are
    bf16 (halves their descriptor cost) and are deferred two bodies in
    the timing loop so their wait on the epilogue never head-of-line
    blocks later stream DMAs in the queue FIFOs.
  * Three sequential NEFF launches: (1) deg -> dinv, x*dinv, (2) layer-1
    aggregation -> h -> v*dinv, (3) layer-2 aggregation -> output.
    Between launches the host only gathers returned per-node arrays into
    per-edge streams (index-space data movement, no float math).

Measured (repeat-delta, 8 cores): deg ~4.5-4.8us, layer1 ~14.7us,
layer2 ~6.7-6.8us; full ~25.9-26.4us vs 27.0us baseline.

KEY EMPIRICAL FINDING for future sessions: per-body time on this stack
is ADDITIVE — body ~= DVE element-cycles + DMA bytes/rate, for every
configuration tried.  Engine overlap does not materialize: merging all
of layer1's 6 stream DMAs into 2 mega-DMAs (one [ew|xs0|xs1] tensor per
block) changed nothing (14.71 vs 14.74us), and GMULT=16/24 (5x fewer
tensor_reduce instructions, +10%/+27% padding) scaled time with PADDED
ELEMENT COUNT, not instruction count (27.4/29.9us).  So only two levers
exist: total streamed bytes and total DVE element-cycles; both are at
their floor for this layout (12B/slot bf16, 1.125 DVE cyc/elem, 12%
padding forced by the g=8 tree groups).  Suspected cause: SBUF
bandwidth contention between SDMA writes and DVE packed-mode reads (or
vNC virtualization serializing engines).

Probed and rejected: fp8 streams (rel err 2-7e-2, over budget; u8
fixed-point ew is byte-neutral after the 1x-mode conversion penalty),
CCE accumulate-multiply DMA (walrus: mult unsupported in Copy mode),
gpsimd SWDGE DMA inside For_i (walrus codegen error), gpsimd
tensor_reduce (only supports partition-axis C/XYZWC, not free-dim X),
PE-based segment reduce (PSUM output lands in nodes-per-column
partitions, starving the epilogue), a 3rd bf16 tree level (stride-1
slices drop DVE to 1x: wash), custom fused DVE dot-pair op (Spec DSL is
elementwise-only, fused ops run 1x), NBLK=3/4 (worse), GMULT=16/24
(worse), per-run two-stage tree enabling GMULT=4 (-5.6% padding but
the strided per-run stages lose the 2x packed mode: 29.3us, reverted).
"""

import sys

sys.path.insert(0, "/opt/trn_rl_repo")

import numpy as np
import ml_dtypes

import concourse.bass as bass
import concourse.tile as tile
from concourse import mybir
from concourse.bass_utils import run_bass_kernel_spmd

BF16 = ml_dtypes.bfloat16

N = 100000
E = 3200000
D = 2
HID = 16
NCORE = 8
TPC = 98                      # stripes == node tiles per core
NPAD = TPC * 1024             # 100352
GMULT = 8                     # stripe slot width rounded up to this
NBLK = 2                      # stream DMA blocks per sweep (pipeline depth)


def _split_multi_waits(nc):
    """This toolchain's walrus encodes at most one sync-wait per instruction.
    Hoist extra waits onto fresh single-wait NoOps placed just before."""
    ctr = 0
    for fn in nc.m.functions:
        for bb in fn.blocks:
            insts = list(bb.instructions)
            if not any(
                i.sync_info is not None and len(i.sync_info.on_wait or []) > 1
                for i in insts
            ):
                continue
            new = []
            for inst in insts:
                si = inst.sync_info
                if si is not None and len(si.on_wait or []) > 1:
                    waits = list(si.on_wait)
                    for w in waits[:-1]:
                        ctr += 1
                        new.append(
                            mybir.InstNoOp(
                                name=f"wsplit-{ctr}",
                                engine=inst.engine,
                                sync_info=mybir.SyncInfo(on_wait=[w], on_update=[]),
                                bass_nofuse=True,
                            )
                        )
                    si.on_wait = [waits[-1]]
                new.append(inst)
            bb.instructions = new
    return ctr


def _preprocess(edge_index, edge_weight):
    """Append self-loops, degree-sort nodes, assign each node a
    partition-row slot range, and scatter edge weight / src index into the
    per-core slot streams."""
    loop = np.arange(N, dtype=np.int64)
    dst = np.concatenate([edge_index[1].astype(np.int64), loop])
    src = np.concatenate([edge_index[0].astype(np.int64), loop])
    ew = np.concatenate([edge_weight.astype(np.float32),
                         np.ones(N, np.float32)])
    ne = len(dst)

    deg = np.bincount(dst, minlength=NPAD)
    order = np.argsort(deg, kind="stable")       # newpos -> orig id
    newpos = np.empty(NPAD, np.int64)
    newpos[order] = np.arange(NPAD)

    counts_new = deg[order]                      # per-newpos degree
    smax = counts_new.reshape(TPC, 1024).max(axis=1)
    G = np.maximum(GMULT, ((smax + GMULT - 1) // GMULT) * GMULT).astype(np.int64)
    offs = np.zeros(TPC + 1, np.int64)
    np.cumsum(G, out=offs[1:])
    CS = int(offs[-1])

    nd = newpos[dst]
    start = np.zeros(NPAD + 1, np.int64)
    np.cumsum(counts_new, out=start[1:])
    perm = np.argsort(nd, kind="stable")
    r = np.empty(ne, np.int64)
    r[perm] = np.arange(ne) - start[nd[perm]]    # rank of edge within its dst

    s = nd >> 10
    w = nd & 1023
    c = w >> 7
    p = w & 127
    flat = (c * 128 + p) * CS + offs[s] + r

    ew_flat = np.zeros(NCORE * 128 * CS, np.float32)
    src_flat = np.zeros(NCORE * 128 * CS, np.int64)
    ew_flat[flat] = ew
    src_flat[flat] = src

    # DMA blocks: consecutive stripes, NBLK roughly equal-column blocks;
    # per-block runs of stripes sharing G (one tensor_reduce per run).
    blk_cols = (CS + NBLK - 1) // NBLK
    blocks = []
    t0, cols = 0, 0
    for t in range(TPC):
        cols += int(G[t])
        if cols >= blk_cols or t == TPC - 1:
            runs = []
            ro = 0
            for tt in range(t0, t + 1):
                g = int(G[tt])
                if runs and runs[-1][2] == g:
                    runs[-1] = (runs[-1][0], runs[-1][1] + 1, g, runs[-1][3])
                else:
                    runs.append((tt, 1, g, ro))
                ro += g
            blocks.append((t0, t + 1 - t0, int(offs[t0]), cols, runs))
            t0, cols = t + 1, 0

    return dict(G=G, offs=offs, CS=CS, blocks=blocks, order=order,
                ew=ew_flat, src=src_flat)


def _stream_blocks(sched, arrflat, prefix, dtype):
    """Per-core dicts of per-DMA-block contiguous stream arrays."""
    CS = sched["CS"]
    a = arrflat.reshape(NCORE, 128, CS)
    out = []
    for c in range(NCORE):
        d = {}
        for bi, (t0, ntb, c0, bc, runs) in enumerate(sched["blocks"]):
            d[f"{prefix}{bi}"] = np.ascontiguousarray(
                a[c, :, c0:c0 + bc]).astype(dtype)
        out.append(d)
    return out


def _to_core_nodes(val_new, dtype):
    """[NPAD] array in newpos space -> per-core [128, TPC]
    (newpos = s*1024 + c*128 + p)."""
    a = val_new.reshape(TPC, NCORE, 128)
    return [np.ascontiguousarray(a[:, c, :].T).astype(dtype) for c in range(NCORE)]


def _from_core_nodes(parts):
    full = np.empty((TPC, NCORE, 128), np.float32)
    for c in range(NCORE):
        full[:, c, :] = np.asarray(parts[c], np.float32).T
    return full.reshape(NPAD)


def _build_sweep(mode, sched, reps=1, variant=None, unroll=16,
                 skip_b1=True, skip_b2=True, npos=HID,
                 flush_bf16=True, flush_split=False, gp_reduce=0):
    """Build the Bass program for one sweep. mode in {deg, layer1, layer2}.
    reps>1 wraps `reps` copies of the (idempotent) body in a hardware For_i
    loop, `unroll` bodies per trip — used only for timing measurements.
    variant (timing experiments only): 'dmaonly' = stream DMA without
    compute, 'reduceonly' = compute without stream DMA."""
    from contextlib import ExitStack

    CS = sched["CS"]
    blocks = sched["blocks"]
    BCMAX = max(b[3] for b in blocks)
    f32 = mybir.dt.float32
    bf = mybir.dt.bfloat16

    nc = bass.Bass("TRN2", target_bir_lowering=False, debug=False,
                   num_devices=NCORE)

    def din(name, shape, dtype=f32):
        return nc.dram_tensor(name, shape, dtype, kind="ExternalInput").ap()

    def dout(name, shape, dtype=f32):
        return nc.dram_tensor(name, shape, dtype, kind="ExternalOutput").ap()

    def din_blocks(prefix):
        return [din(f"{prefix}{bi}", [128, b[3]], bf)
                for bi, b in enumerate(blocks)]

    if mode == "deg":
        ew_d = din_blocks("ew")
        xn_d = [din(f"xn{f}", [128, TPC]) for f in range(D)]
        deg_out = dout("degout", [128, (D + 1) * TPC],
                       mybir.dt.bfloat16 if flush_bf16 else f32)
    elif mode == "layer1":
        # mega-streams: one DMA per block moves [ew | xs0 | xs1]
        m1_d = [din(f"m1_{bi}", [128, 3 * b[3]], bf)
                for bi, b in enumerate(blocks)]
        dinv_d = din("dinvn", [128, TPC])
        w1x_d = [din(f"w1x{f}b", [128, HID * TPC], bf) for f in range(D)]
        b1_d = din("b1b", [128, HID], bf)
        vt_out = dout("vtout", [128, TPC],
                      mybir.dt.bfloat16 if flush_bf16 else f32)
    else:
        m2_d = [din(f"m2_{bi}", [128, 2 * b[3]], bf)
                for bi, b in enumerate(blocks)]
        dinv_d = din("dinvn", [128, TPC])
        b2_d = din("b2b", [128, 1])
        y_out = dout("yout", [128, TPC],
                     mybir.dt.bfloat16 if flush_bf16 else f32)

    with tile.TileContext(nc) as tc, ExitStack() as ctx:
        nbuf = 3 if mode == "layer1" else 6
        const = ctx.enter_context(tc.tile_pool(name="const", bufs=1))
        sp = ctx.enter_context(tc.tile_pool(name="streams", bufs=nbuf))
        wp = ctx.enter_context(tc.tile_pool(name="work", bufs=nbuf))
        accp = ctx.enter_context(tc.tile_pool(name="acc", bufs=2))

        if mode == "deg":
            xN = []
            for f in range(D):
                t_ = const.tile([128, TPC], f32, tag=f"xn{f}")
                nc.sync.dma_start(t_[:], xn_d[f][:])
                xN.append(t_)
        elif mode == "layer1":
            dinvN = const.tile([128, TPC], f32)
            nc.sync.dma_start(dinvN[:], dinv_d[:])
            w1x = []
            for f in range(D):
                t_ = const.tile([128, HID * TPC], bf, tag=f"w1x{f}")
                nc.sync.dma_start(t_[:], w1x_d[f][:])
                w1x.append(t_)
            b1_sb = const.tile([128, HID], bf)
            nc.sync.dma_start(b1_sb[:], b1_d[:])
        else:
            dinvN = const.tile([128, TPC], f32)
            nc.sync.dma_start(dinvN[:], dinv_d[:])
            b2_sb = const.tile([128, 1], f32)
            nc.sync.dma_start(b2_sb[:], b2_d[:])

        nF = D if mode == "layer1" else 1

        def _reduce_2stage(m_t, bc, runs, dst_agg, s1tag, red_eng=None):
            # Pair-add tree: TensorReduce has no fast DVE mode (1x), but
            # TensorTensor adds on packed bf16 sub-slices run at 2x.  Three
            # tree levels collapse each 8-slot group to 1 partial (G is a
            # multiple of 8, so g//4 is always even); a final narrow f32
            # TensorReduce finishes per G-run.
            u = wp.tile([128, BCMAX // 2], bf, tag=s1tag + "u")
            mv = m_t[:, 0:bc].rearrange("p (q g) -> p q g", g=8)
            nc.vector.tensor_tensor(
                u[:, 0:bc // 2].rearrange("p (q h) -> p q h", h=4),
                mv[:, :, 0:4], mv[:, :, 4:8], mybir.AluOpType.add)
            w = wp.tile([128, BCMAX // 4], bf, tag=s1tag + "w")
            uv = u[:, 0:bc // 2].rearrange("p (q h) -> p q h", h=4)
            nc.vector.tensor_tensor(
                w[:, 0:bc // 4].rearrange("p (q h) -> p q h", h=2),
                uv[:, :, 0:2], uv[:, :, 2:4], mybir.AluOpType.add)
            for (tt, nt, g, ro) in runs:
                (red_eng or nc.vector).tensor_reduce(
                    dst_agg[:, tt:tt + nt],
                    w[:, ro // 4:ro // 4 + nt * (g // 4)].rearrange(
                        "p (t q) -> p t q", q=g // 4),
                    mybir.AxisListType.X, mybir.AluOpType.add)

        qrr = [0]

        def qnext():
            # round-robin full-stream DMAs across the two HWDGE queues
            q = (nc.sync, nc.scalar)[qrr[0] % 2]
            qrr[0] += 1
            return q

        def stream_in(t_, d_, bc):
            if variant == "reduceonly":
                qnext().dma_start(t_[:, 0:4], d_[:, 0:4])
            else:
                qnext().dma_start(t_[:, 0:bc], d_[:])

        def body(flush_prev=None):
            # deferred-out protocol: the previous body's output DMA is
            # emitted right after this body's first-block input DMAs, so
            # it never sits in front of stream DMAs in an HWDGE FIFO.
            flushes = []
            agg = [accp.tile([128, TPC], f32, tag=f"agg{f}", name=f"agg{f}")
                   for f in range(nF)]
            for bi, (t0, ntb, c0, bc, runs) in enumerate(blocks):
                if mode == "deg":
                    ew_t = sp.tile([128, BCMAX], bf, tag="ew")
                    stream_in(ew_t, ew_d[bi], bc)
                elif mode == "layer1":
                    meg = sp.tile([128, 3 * BCMAX], bf, tag="meg")
                    stream_in(meg, m1_d[bi], 3 * bc)
                else:
                    meg = sp.tile([128, 2 * BCMAX], bf, tag="meg")
                    stream_in(meg, m2_d[bi], 2 * bc)
                if bi == 0 and flush_prev:
                    for f_ in flush_prev:
                        f_()
                if variant == "dmaonly":
                    continue
                if mode == "deg":
                    _reduce_2stage(ew_t, bc, runs, agg[0], "s1a")
                elif mode == "layer1":
                    ew_s = meg[:, 0:bc]
                    xs_t = [meg[:, bc:2 * bc], meg[:, 2 * bc:3 * bc]]
                    # fused two-feature tree over one wide tile; the
                    # [bc:BCMAX] gaps hold garbage that no reduce reads.
                    mm = wp.tile([128, 2 * BCMAX], bf, tag="mm")
                    nc.vector.tensor_mul(mm[:, 0:bc], ew_s, xs_t[0])
                    nc.vector.tensor_mul(mm[:, BCMAX:BCMAX + bc],
                                         ew_s, xs_t[1])
                    uu = wp.tile([128, BCMAX], bf, tag="uu")
                    mv = mm.rearrange("p (q g) -> p q g", g=8)
                    nc.vector.tensor_tensor(
                        uu.rearrange("p (q h) -> p q h", h=4),
                        mv[:, :, 0:4], mv[:, :, 4:8], mybir.AluOpType.add)
                    ww = wp.tile([128, BCMAX // 2], bf, tag="ww")
                    uv = uu.rearrange("p (q h) -> p q h", h=4)
                    nc.vector.tensor_tensor(
                        ww.rearrange("p (q h) -> p q h", h=2),
                        uv[:, :, 0:2], uv[:, :, 2:4], mybir.AluOpType.add)
                    use_gp = gp_reduce >= 2 or (gp_reduce == 1 and bi % 2 == 1)
                    red = nc.gpsimd if use_gp else nc.vector
                    for f in range(D):
                        w_f = ww[:, f * (BCMAX // 4):(f + 1) * (BCMAX // 4)]
                        for (tt, nt, g, ro) in runs:
                            red.tensor_reduce(
                                agg[f][:, tt:tt + nt],
                                w_f[:, ro // 4:ro // 4 + nt * (g // 4)]
                                .rearrange("p (t q) -> p t q", q=g // 4),
                                mybir.AxisListType.X, mybir.AluOpType.add)
                else:
                    m0 = wp.tile([128, BCMAX], bf, tag="m0")
                    nc.vector.tensor_mul(m0[:, 0:bc], meg[:, 0:bc],
                                         meg[:, bc:2 * bc])
                    _reduce_2stage(m0, bc, runs, agg[0], "s1a")

            if variant in ("dmaonly", "reduceonly", "noepi"):
                return
            # ---- epilogue (self-loop slots make agg complete: deg sweep
            # yields deg+1; layer sweeps include the dinv*val self term) ----
            if mode == "deg":
                sq = wp.tile([128, TPC], f32, tag="sq")
                nc.scalar.activation(sq, agg[0],
                                     mybir.ActivationFunctionType.Sqrt)
                pk = wp.tile([128, (D + 1) * TPC],
                             bf if flush_bf16 else f32, tag="pk")
                if flush_bf16:
                    di = wp.tile([128, TPC], f32, tag="di")
                    nc.vector.reciprocal(di, sq)
                    nc.vector.tensor_copy(pk[:, 0:TPC], di)
                else:
                    nc.vector.reciprocal(pk[:, 0:TPC], sq)
                for f in range(D):
                    nc.vector.tensor_mul(pk[:, (1 + f) * TPC:(2 + f) * TPC],
                                         xN[f],
                                         di if flush_bf16 else pk[:, 0:TPC])
                if flush_split:
                    h = (D + 1) * TPC // 2
                    flushes.append(lambda pk=pk, h=h: qnext().dma_start(
                        deg_out[:, 0:h], pk[:, 0:h]))
                    flushes.append(lambda pk=pk, h=h: qnext().dma_start(
                        deg_out[:, h:], pk[:, h:]))
                else:
                    flushes.append(lambda pk=pk: qnext().dma_start(
                        deg_out[:], pk[:]))
            elif mode == "layer1":
                zb = []
                for f in range(D):
                    zb_ = wp.tile([128, TPC], bf, tag=f"zb{f}")
                    nc.vector.tensor_mul(zb_, agg[f], dinvN)
                    zb.append(zb_)
                # h layout [128, (j t)]: hidden-unit major; weights arrive
                # pre-materialized in the same layout so every product runs
                # in the 2x packed-bf16 DVE mode.
                hA = wp.tile([128, TPC * HID], bf, tag="hA")
                nc.vector.tensor_tensor(
                    hA.rearrange("p (j t) -> p j t", j=HID),
                    zb[0].unsqueeze(1).broadcast_to([128, HID, TPC]),
                    w1x[0].rearrange("p (j t) -> p j t", j=HID),
                    mybir.AluOpType.mult)
                hB = wp.tile([128, TPC * HID], bf, tag="hB")
                nc.vector.tensor_tensor(
                    hB.rearrange("p (j t) -> p j t", j=HID),
                    zb[1].unsqueeze(1).broadcast_to([128, HID, TPC]),
                    w1x[1].rearrange("p (j t) -> p j t", j=HID),
                    mybir.AluOpType.mult)
                nc.vector.tensor_add(hA, hA, hB)
                if not skip_b1:
                    nc.vector.tensor_tensor(
                        hA.rearrange("p (j t) -> p j t", j=HID),
                        hA.rearrange("p (j t) -> p j t", j=HID),
                        b1_sb.unsqueeze(2).broadcast_to([128, HID, TPC]),
                        mybir.AluOpType.add)
                if npos > 0:
                    nc.vector.tensor_scalar_max(hA[:, 0:npos * TPC],
                                                hA[:, 0:npos * TPC], 0.0)
                if npos < HID:
                    nc.vector.tensor_scalar_min(hA[:, npos * TPC:],
                                                hA[:, npos * TPC:], 0.0)
                T8 = 8 * TPC
                va = wp.tile([128, T8], bf, tag="va")
                nc.vector.tensor_add(va, hA[:, 0:T8], hA[:, T8:2 * T8])
                vb = wp.tile([128, T8 // 2], bf, tag="vb")
                nc.vector.tensor_add(vb, va[:, 0:T8 // 2], va[:, T8 // 2:T8])
                vc = wp.tile([128, T8 // 4], bf, tag="vc")
                nc.vector.tensor_add(vc, vb[:, 0:T8 // 4],
                                     vb[:, T8 // 4:T8 // 2])
                vd = wp.tile([128, TPC], f32, tag="vd")
                nc.vector.tensor_add(vd, vc[:, 0:TPC], vc[:, TPC:2 * TPC])
                vt = wp.tile([128, TPC], bf if flush_bf16 else f32,
                             tag="vt")
                nc.vector.tensor_mul(vt, vd, dinvN)
                flushes.append(lambda vt=vt: qnext().dma_start(vt_out[:], vt[:]))
            else:
                y = wp.tile([128, TPC], bf if flush_bf16 else f32, tag="y")
                nc.vector.tensor_mul(y, agg[0], dinvN)
                if not skip_b2:
                    nc.vector.tensor_scalar(y, y, b2_sb[:, 0:1], None,
                                            mybir.AluOpType.add)
                flushes.append(lambda y=y: qnext().dma_start(y_out[:], y[:]))
            return flushes

        if reps > 1:
            # Flushes are deferred TWO bodies so an output DMA's wait on
            # its epilogue is already satisfied when the FIFO reaches it
            # (no head-of-line blocking of later stream DMAs).
            assert reps % unroll == 0
            f1, f2 = None, None
            with tc.For_i(0, reps // unroll, 1):
                for _ in range(unroll):
                    f1, f2 = body(f2), f1
            for f_ in (f2 or []) + (f1 or []):
                f_()
        else:
            for f_ in body(None) or []:
                f_()

    _split_multi_waits(nc)
    return nc



def _merge_blocks(sched, parts_cs, prefix):
    """Concatenate per-block stream arrays column-wise into one mega
    tensor per block: parts_cs is a list of per-core block dicts sharing
    block indices; result[c][f"{prefix}{bi}"] = [128, k*bc]."""
    out = []
    nb = len(sched["blocks"])
    for c in range(len(parts_cs[0])):
        d = {}
        for bi in range(nb):
            d[f"{prefix}{bi}"] = np.ascontiguousarray(np.concatenate(
                [list(p[c].values())[bi] for p in parts_cs], axis=1))
        out.append(d)
    return out

def _rep_bf16(vec):
    return np.ascontiguousarray(
        np.tile(np.asarray(vec, np.float32).reshape(1, -1), (128, 1))
    ).astype(BF16)


def kernel(x, edge_index, edge_weight, W1, b1, W2, b2):
    x = np.asarray(x, np.float32)
    edge_index = np.asarray(edge_index)
    edge_weight = np.asarray(edge_weight, np.float32)
    W1 = np.asarray(W1, np.float32)
    b1 = np.asarray(b1, np.float32)
    W2 = np.asarray(W2, np.float32)
    b2 = np.asarray(b2, np.float32)
    skip_b1 = not np.any(b1 != 0)
    skip_b2 = not np.any(b2 != 0)

    pp = _preprocess(edge_index, edge_weight)
    order = pp["order"]

    ew_cs = _stream_blocks(pp, pp["ew"], "ew", BF16)

    xfull = np.zeros((NPAD, D), np.float32)
    xfull[:N] = x
    xnew = xfull[order]                          # newpos layout
    xn_cs = [_to_core_nodes(xnew[:, f], np.float32) for f in range(D)]

    # ---- NEFF 1: deg+1 -> dinv, x*dinv ----
    nc1 = _build_sweep("deg", pp)
    in1 = [dict(ew_cs[c], xn0=xn_cs[0][c], xn1=xn_cs[1][c])
           for c in range(NCORE)]
    r1 = run_bass_kernel_spmd(nc1, in1, core_ids=list(range(NCORE)))
    pk = [np.asarray(r1.results[c]["degout"], np.float32) for c in range(NCORE)]
    dinv_new = _from_core_nodes([p[:, 0:TPC] for p in pk])
    xt_new = [_from_core_nodes([p[:, (1 + f) * TPC:(2 + f) * TPC] for p in pk])
              for f in range(D)]

    # ---- host glue: per-edge (x*dinv)[src] streams ----
    xt_orig = np.empty((NPAD, D), np.float32)
    for f in range(D):
        xt_orig[order, f] = xt_new[f]
    xs_cs = [_stream_blocks(pp, xt_orig[pp["src"], f], f"xs{f}_", BF16)
             for f in range(D)]
    dinv_n = _to_core_nodes(dinv_new, np.float32)

    # fold W2 into W1 (g_j = h_j * w2_j); order non-negative-w2 units first
    w2v = W2[:, 0]
    perm = np.argsort(w2v < 0, kind="stable")
    npos = int((w2v >= 0).sum())
    W1p = (W1 * w2v[None, :])[:, perm]
    b1p = (b1 * w2v)[perm]
    w1x = [_rep_bf16(np.repeat(W1p[f], TPC)) for f in range(D)]
    b1b = _rep_bf16(b1p)
    b2b = np.full((128, 1), float(b2[0]), np.float32)

    # ---- NEFF 2: layer 1 -> v*dinv ----
    nc2 = _build_sweep("layer1", pp, skip_b1=skip_b1, skip_b2=skip_b2,
                       npos=npos)
    m1_cs = _merge_blocks(pp, [ew_cs, xs_cs[0], xs_cs[1]], "m1_")
    in2 = [dict(m1_cs[c],
                dinvn=dinv_n[c], w1x0b=w1x[0], w1x1b=w1x[1], b1b=b1b)
           for c in range(NCORE)]
    r2 = run_bass_kernel_spmd(nc2, in2, core_ids=list(range(NCORE)))
    vt_new = _from_core_nodes([r2.results[c]["vtout"] for c in range(NCORE)])

    # ---- host glue: (v*dinv)[src] stream ----
    vt_orig = np.empty(NPAD, np.float32)
    vt_orig[order] = vt_new
    vs_cs = _stream_blocks(pp, vt_orig[pp["src"]], "vs", BF16)
    dinv_n2 = dinv_n

    # ---- NEFF 3: layer 2 -> output ----
    nc3 = _build_sweep("layer2", pp, skip_b1=skip_b1, skip_b2=skip_b2)
    m2_cs = _merge_blocks(pp, [ew_cs, vs_cs], "m2_")
    in3 = [dict(m2_cs[c], dinvn=dinv_n2[c], b2b=b2b)
           for c in range(NCORE)]
    r3 = run_bass_kernel_spmd(nc3, in3, core_ids=list(range(NCORE)))
    y_new = _from_core_nodes([r3.results[c]["yout"] for c in range(NCORE)])

    y_orig = np.empty(NPAD, np.float32)
    y_orig[order] = y_new
    return y_orig[:N, None].astype(np.float32)

